# revision 1
# baseline (speedup 1.0000x reference)
"""Trainium2 Bass kernel: PSI block (LN1 -> sigmoid-gated value -> chunked
normalized cumsum -> residual -> LN2 -> exact-gelu FFN -> residual).

Sharding: 32768 tokens split into 8 contiguous 4096-token shards (chunk- and
batch-boundary aligned), one per NeuronCore; dim-sized weights replicated.

Per core, two passes:
  pass A (32 token-tiles of 128): LN1 stats (ACT Square accum + DVE reduce,
    Newton-iteration rsqrt on DVE to stay on a single ACT table), z.T built by
    matmuls against diag(rstd), bf16 gate/value matmuls, sigmoid, chunked
    cumsum via block-triangular matmul, mem = csum(g*v)/(csum(g)+1e-6),
    x2 = x + mem -> DRAM scratch, LN2 stats stored for pass B.
  pass B (8 macros of 512 tokens): h2.T via PE transposes, FFN1 (bf16,
    output transposed [f, tok]), exact gelu via Erf, FFN2 accumulating into
    transposed PSUM out with fp32 x2-residual transposes folded into the same
    accumulation, drain [dout, tok] -> DRAM; host transposes back.
"""

import sys

sys.path.insert(0, "/opt/trn_rl_repo")

import numpy as np
import ml_dtypes
from contextlib import ExitStack

B, S, D, CHUNK = 4, 8192, 768, 64
NCORES = 8
TOTAL = B * S              # 32768 tokens
TPC = TOTAL // NCORES      # 4096 tokens per core
KD = D // 128              # 6 k-blocks over D
H = 4 * D                  # 3072 FFN hidden
KH = H // 128              # 24 k-blocks over H
MACRO = 512                # pass-B token macro
INV_SQRT2 = 0.7071067811865476


def build(T=TPC, erf_ok=True, gbias=False, vbias=False, f1bias=False, f2bias=False,
          reps=1):
    import concourse.bass as bass
    import concourse.bacc as bacc
    import concourse.tile as tile
    from concourse import mybir

    F32 = mybir.dt.float32
    BF16 = mybir.dt.bfloat16
    I32 = mybir.dt.int32
    AF = mybir.ActivationFunctionType
    ALU = mybir.AluOpType
    PSUM = bass.MemorySpace.PSUM
    DRAM = bass.MemorySpace.DRAM
    ts = bass.ts

    NT = T // 128
    NM = T // MACRO
    NS = MACRO // 128
    any_bias = gbias or vbias or f1bias or f2bias

    nc = bacc.Bacc(None, target_bir_lowering=False, debug=False)

    gv_b = gbias or vbias
    x_d = nc.dram_tensor("x", [T, D], F32, kind="ExternalInput")
    wgv_d = nc.dram_tensor("wgv", [KD, 128, 2 * D], BF16, kind="ExternalInput")
    w1_d = nc.dram_tensor("w1", [KD, 128, H], BF16, kind="ExternalInput")
    w2_d = nc.dram_tensor("w2", [KH, 128, D], BF16, kind="ExternalInput")
    u_d = nc.dram_tensor("u", [128, 128], BF16, kind="ExternalInput")
    idb_d = nc.dram_tensor("idb", [128, 128], BF16, kind="ExternalInput")
    idf_d = nc.dram_tensor("idf", [128, 128], F32, kind="ExternalInput")
    bgv_d = nc.dram_tensor("bgv", [1, 2 * D], BF16, kind="ExternalInput") if gv_b else None
    b1_d = nc.dram_tensor("b1", [1, H], BF16, kind="ExternalInput") if f1bias else None
    b2_d = nc.dram_tensor("b2", [1, D], BF16, kind="ExternalInput") if f2bias else None
    outT_d = nc.dram_tensor("outT", [D, T], F32, kind="ExternalOutput")

    with tile.TileContext(nc) as tc, ExitStack() as ctx:
        dram = ctx.enter_context(tc.tile_pool(name="dram", bufs=1, space=DRAM))
        x2_d = dram.tile([T, D], F32, name="x2scratch")

        const = ctx.enter_context(tc.tile_pool(name="const", bufs=1))
        wgv_sb = const.tile([128, KD, 2 * D], BF16, tag="wgv")
        w1_sb = const.tile([128, KD, H], BF16, tag="w1")
        for k in range(KD):
            nc.sync.dma_start(wgv_sb[:, k, :], wgv_d[k])
            nc.sync.dma_start(w1_sb[:, k, :], w1_d[k])
        w2_sb = const.tile([128, KH, D], BF16, tag="w2")
        for k in range(KH):
            nc.sync.dma_start(w2_sb[:, k, :], w2_d[k])
        u_sb = const.tile([128, 128], BF16, tag="u")
        nc.sync.dma_start(u_sb[:], u_d[:])
        idb_sb = const.tile([128, 128], BF16, tag="idb")
        nc.sync.dma_start(idb_sb[:], idb_d[:])
        idf_sb = const.tile([128, 128], F32, tag="idf")
        nc.sync.dma_start(idf_sb[:], idf_d[:])
        rstd2_all = const.tile([128, NT], F32, tag="rstd2")
        nmr2_all = const.tile([128, NT], F32, tag="nmr2")
        eps_sb = const.tile([128, 1], F32, tag="eps")
        nc.vector.memset(eps_sb[:], 1e-6)
        if gv_b:
            bgv_sb = const.tile([1, 2 * D], BF16, tag="bgv")
            nc.sync.dma_start(bgv_sb[:], bgv_d[:])
        if f1bias:
            b1_sb = const.tile([1, H], BF16, tag="b1")
            nc.sync.dma_start(b1_sb[:], b1_d[:])
        if f2bias:
            b2_sb = const.tile([1, D], BF16, tag="b2")
            nc.sync.dma_start(b2_sb[:], b2_d[:])
        if any_bias:
            ones_sb = const.tile([1, MACRO], BF16, tag="ones")
            nc.vector.memset(ones_sb[:], 1.0)

        def ln_stats(pool, tag, src):
            """Row stats of src [128, D] f32: returns (nmu, v) = (-mean, var+eps)."""
            sqscr = pool.tile([128, D], BF16, tag="sqscr", bufs=2, name="sqscr")
            sqs = pool.tile([128, 1], F32, tag=tag + "_sqs", bufs=2, name="sqs")
            nc.scalar.activation(sqscr[:], src[:], AF.Square, accum_out=sqs[:])
            xs = pool.tile([128, 1], F32, tag=tag + "_xs", bufs=2, name="xs")
            nc.vector.tensor_reduce(xs[:], src[:], mybir.AxisListType.X, ALU.add)
            nmu = pool.tile([128, 1], F32, tag=tag + "_nmu", bufs=2, name="nmu")
            nc.vector.tensor_scalar(nmu[:], xs[:], -1.0 / D, None, op0=ALU.mult)
            v = pool.tile([128, 1], F32, tag=tag + "_v", bufs=2, name="v")
            nc.vector.tensor_scalar(v[:], sqs[:], 1.0 / D, 1e-5, op0=ALU.mult, op1=ALU.add)
            m2 = pool.tile([128, 1], F32, tag=tag + "_m2", bufs=2, name="m2")
            nc.vector.tensor_mul(m2[:], nmu[:], nmu[:])
            nc.vector.tensor_sub(v[:], v[:], m2[:])
            return nmu, v

        def newton_rsqrt(pool, tag, v, out_ap=None):
            """y ~ rsqrt(v) for v [128,1] f32 > 0; quake seed + 2 NR iters on DVE."""
            y = pool.tile([128, 1], F32, tag=tag + "_y", bufs=2, name="y")
            a = pool.tile([128, 1], F32, tag=tag + "_a", bufs=2, name="a")
            nc.vector.tensor_scalar(
                y[:].bitcast(I32), v[:].bitcast(I32), 1, -1,
                op0=ALU.logical_shift_right, op1=ALU.bitwise_xor,
            )
            nc.vector.tensor_scalar(
                y[:].bitcast(I32), y[:].bitcast(I32), 0x5F3759E0, None, op0=ALU.add
            )
            for it in range(2):
                nc.vector.tensor_mul(a[:], y[:], y[:])
                nc.vector.tensor_mul(a[:], a[:], v[:])
                nc.vector.tensor_scalar(a[:], a[:], -0.5, 1.5, op0=ALU.mult, op1=ALU.add)
                dst = out_ap if (it == 1 and out_ap is not None) else y[:]
                nc.vector.tensor_mul(dst, y[:], a[:])
            return y

        # ---------------- pass A ----------------
        def pass_a():
            with tc.tile_pool(name="pa", bufs=1) as pa, \
                    tc.tile_pool(name="psa", bufs=1, space=PSUM) as psa:
                run_pass_a(pa, psa)

        def run_pass_a(pa, psa):
            xs, hus, lnts = {}, {}, {}

            def stage_dma(t):
                x_sb = pa.tile([128, D], F32, tag="x", bufs=3, name="x_sb")
                nc.sync.dma_start(x_sb[:], x_d[128 * t:128 * (t + 1), :])
                xs[t] = x_sb

            def stage_stats(t):
                x_sb = xs[t]
                nmu, v = ln_stats(pa, "s1", x_sb)
                rstd = newton_rsqrt(pa, "n1", v)
                nmr1 = pa.tile([128, 1], F32, tag="nmr1", bufs=2, name="nmr1")
                nc.vector.tensor_mul(nmr1[:], nmu[:], rstd[:])
                # normalized x in bf16: (x - mu) * rstd in one DVE op
                hu = pa.tile([128, D], BF16, tag="hu", bufs=2, name="hu")
                nc.vector.tensor_scalar(hu[:], x_sb[:], rstd[:], nmr1[:],
                                        op0=ALU.mult, op1=ALU.add)
                hus[t] = hu

            def stage_lnT(t):
                # z.T blocks via pure PE transposes (no Ldweights, bf16 PSUM)
                hu = hus.pop(t)
                lnT_ps = psa.tile([128, KD, 128], BF16, tag="lnT", bufs=2,
                                  padded_shape=[128, 8, 128], name="lnT_ps")
                for k in range(KD):
                    nc.tensor.transpose(lnT_ps[:, k, :], hu[:, ts(k, 128)], idb_sb[:])
                lnT = pa.tile([128, KD, 128], BF16, tag="lnT", bufs=2, name="lnT")
                nc.scalar.copy(lnT[:], lnT_ps[:])
                lnts[t] = lnT

            stage_dma(0)
            stage_stats(0)
            stage_lnT(0)
            if NT > 1:
                stage_dma(1)
            for t in range(NT):
                tok0 = 128 * t
                x_sb = xs.pop(t)
                lnT = lnts.pop(t)
                # merged gate|value pre-activations: wgv columns are
                # [g 0:512 | v 0:512 | g 512:768 | v 512:768] -> 3 full banks
                pgv = psa.tile([128, 3, 512], F32, tag="pgv", bufs=1, name="pgv")
                for bb in range(3):
                    mm = [(lnT[:, k, :], wgv_sb[:, k, 512 * bb:512 * (bb + 1)])
                          for k in range(KD)]
                    if gv_b:
                        mm.append((ones_sb[0:1, 0:128],
                                   bgv_sb[0:1, 512 * bb:512 * (bb + 1)]))
                    for i, (l, r) in enumerate(mm):
                        nc.tensor.matmul(pgv[:, bb, :], l, r,
                                         start=(i == 0), stop=(i == len(mm) - 1))
                # gvg packs [g 0:768 | g*v 0:768] (original column order) so the
                # cumsum is 3 full-width matmuls and cs reads stay bank-aligned
                gvg = pa.tile([128, 2 * D], BF16, tag="gvg", bufs=2, name="gvg")
                nc.scalar.activation(gvg[:, 0:512], pgv[:, 0, :], AF.Sigmoid)
                nc.scalar.activation(gvg[:, 512:768], pgv[:, 2, 0:256], AF.Sigmoid)
                nc.vector.tensor_mul(gvg[:, 768:1280], gvg[:, 0:512], pgv[:, 1, :])
                nc.vector.tensor_mul(gvg[:, 1280:1536], gvg[:, 512:768],
                                     pgv[:, 2, 256:512])
                # prefetch + LN1 stats for upcoming tiles while PE chews on t
                if t + 2 < NT:
                    stage_dma(t + 2)
                if t + 1 < NT:
                    stage_stats(t + 1)
                # chunked cumsum along tokens (partition dim) via triangular
                # matmul; cs = [csum(g) 0:768 | csum(g*v) 0:768]
                cs = psa.tile([128, 3, 512], F32, tag="cs", bufs=1, name="cs")
                for bb in range(3):
                    nc.tensor.matmul(cs[:, bb, :], u_sb[:],
                                     gvg[:, 512 * bb:512 * (bb + 1)],
                                     start=True, stop=True)
                if t + 1 < NT:
                    stage_lnT(t + 1)
                den = pa.tile([128, D], F32, tag="den", bufs=2, name="den")
                rcp = pa.tile([128, D], F32, tag="rcp", bufs=2, name="rcp")
                mem = pa.tile([128, D], F32, tag="mem", bufs=2, name="mem")
                x2 = pa.tile([128, D], F32, tag="x2", bufs=3, name="x2")
                nc.scalar.activation(den[:, 0:512], cs[:, 0, :], AF.Identity,
                                     bias=eps_sb[:])
                nc.scalar.activation(den[:, 512:768], cs[:, 1, 0:256], AF.Identity,
                                     bias=eps_sb[:])
                nc.vector.reciprocal_approx_fast(rcp[:], den[:])
                nc.vector.tensor_mul(mem[:, 0:256], rcp[:, 0:256], cs[:, 1, 256:512])
                nc.vector.tensor_mul(mem[:, 256:768], rcp[:, 256:768], cs[:, 2, :])
                nc.vector.tensor_add(x2[:], x_sb[:], mem[:])
                nc.sync.dma_start(x2_d[tok0:tok0 + 128, :], x2[:])
                nmu2, v2 = ln_stats(pa, "s2", x2)
                newton_rsqrt(pa, "n2", v2, out_ap=rstd2_all[:, t:t + 1])
                nc.vector.tensor_mul(nmr2_all[:, t:t + 1], nmu2[:], rstd2_all[:, t:t + 1])

        # ---------------- pass B ----------------
        def pass_b():
            with tc.tile_pool(name="pb", bufs=1) as pb, \
                    tc.tile_pool(name="psb", bufs=1, space=PSUM) as psb:
                run_pass_b(pb, psb)

        def run_pass_b(pb, psb):
            for m in range(NM):
                tok0 = MACRO * m
                outT_ps = psb.tile([128, KD, MACRO], F32, tag="outT", bufs=1,
                                   name="outT_ps")
                h2T = pb.tile([128, KD, MACRO], BF16, tag="h2T", bufs=2, name="h2T")
                x2s_list = []
                for s in range(NS):
                    tm = m * NS + s
                    x2s = pb.tile([128, D], F32, tag="x2s", bufs=NS + 1, name="x2s")
                    x2s_list.append(x2s)
                    nc.sync.dma_start(x2s[:], x2_d[tok0 + 128 * s:tok0 + 128 * (s + 1), :])
                    h2s = pb.tile([128, D], BF16, tag="h2s", bufs=2, name="h2s")
                    nc.vector.tensor_scalar(h2s[:], x2s[:], rstd2_all[:, tm:tm + 1],
                                            nmr2_all[:, tm:tm + 1],
                                            op0=ALU.mult, op1=ALU.add)
                    tps = psb.tile([128, KD, 128], BF16, tag="pt", bufs=2,
                                   padded_shape=[128, 8, 128], name="tps")
                    for k in range(KD):
                        nc.tensor.transpose(tps[:, k, :], h2s[:, ts(k, 128)], idb_sb[:])
                    nc.scalar.copy(h2T[:, :, 128 * s:128 * (s + 1)], tps[:])
                # software-pipelined: FFN1(f) is issued before FFN2(f-1) so the
                # gelu/uT of block f computes while PE runs FFN2 of block f-1.
                prev_uT = None
                for f in range(KH + 1):
                    if f < KH:
                        pT = psb.tile([128, MACRO], F32, tag="pt", bufs=2, name="pT")
                        mm = [(w1_sb[:, k, 128 * f:128 * (f + 1)], h2T[:, k, :])
                              for k in range(KD)]
                        if f1bias:
                            mm.append((b1_sb[0:1, 128 * f:128 * (f + 1)],
                                       ones_sb[0:1, 0:MACRO]))
                        for i, (l, r) in enumerate(mm):
                            nc.tensor.matmul(pT[:], l, r,
                                             start=(i == 0), stop=(i == len(mm) - 1))
                    if f >= 1:
                        fp = f - 1
                        for m2 in range(KD):
                            nc.tensor.matmul(outT_ps[:, m2, :],
                                             w2_sb[:, fp, 128 * m2:128 * (m2 + 1)],
                                             prev_uT[:],
                                             start=(fp == 0), stop=(fp == KH - 1))
                        # fp == 0 opened each bank's group full-width; residual/bias
                        # transposes then accumulate into the open group.
                        if fp == 0:
                            for s in range(NS):
                                for m2 in range(KD):
                                    nc.tensor.matmul(
                                        outT_ps[:, m2, 128 * s:128 * (s + 1)],
                                        x2s_list[s][:, ts(m2, 128)], idf_sb[:],
                                        start=False, stop=False, is_transpose=True)
                            if f2bias:
                                for m2 in range(KD):
                                    nc.tensor.matmul(outT_ps[:, m2, :],
                                                     b2_sb[0:1, 128 * m2:128 * (m2 + 1)],
                                                     ones_sb[0:1, 0:MACRO],
                                                     start=False, stop=False)
                    if f < KH:
                        e_sb = pb.tile([128, MACRO], BF16, tag="e", bufs=2, name="e_sb")
                        nc.scalar.activation(e_sb[:], pT[:],
                                             AF.Erf if erf_ok else AF.Tanh,
                                             scale=INV_SQRT2)
                        uT = pb.tile([128, MACRO], BF16, tag="uT", bufs=3, name="uT")
                        nc.vector.scalar_tensor_tensor(uT[:], e_sb[:], 1.0, pT[:],
                                                       op0=ALU.add, op1=ALU.mult)
                        prev_uT = uT
                for m2 in range(KD):
                    osb = pb.tile([128, MACRO], F32, tag="osb", bufs=2, name="osb")
                    nc.scalar.copy(osb[:], outT_ps[:, m2, :])
                    nc.sync.dma_start(outT_d[128 * m2:128 * (m2 + 1), tok0:tok0 + MACRO],
                                      osb[:])

        for _ in range(reps):
            pass_a()
            pass_b()

    nc.compile()
    return nc


def _fold(inputs):
    f32 = np.float32
    bf16 = ml_dtypes.bfloat16
    n1w = np.asarray(inputs["norm1_w"], f32)
    n1b = np.asarray(inputs["norm1_b"], f32)
    n2w = np.asarray(inputs["norm2_w"], f32)
    n2b = np.asarray(inputs["norm2_b"], f32)
    gW = np.asarray(inputs["gate_W"], f32)
    gb = np.asarray(inputs["gate_b"], f32)
    vW = np.asarray(inputs["value_W"], f32)
    vb = np.asarray(inputs["value_b"], f32)
    W1 = np.asarray(inputs["ffn_W1"], f32)
    b1 = np.asarray(inputs["ffn_b1"], f32)
    W2 = np.asarray(inputs["ffn_W2"], f32)
    b2 = np.asarray(inputs["ffn_b2"], f32)

    bg = (n1b @ gW + gb).astype(bf16).reshape(1, D)
    bv = (n1b @ vW + vb).astype(bf16).reshape(1, D)
    b1f = (n2b @ W1 + b1).astype(bf16).reshape(1, H)
    b2f = b2.astype(bf16).reshape(1, D)
    flags = (bool(bg.any()), bool(bv.any()), bool(b1f.any()), bool(b2f.any()))

    tri = np.triu(np.ones((CHUNK, CHUNK), f32))
    u = np.zeros((128, 128), f32)
    for c in range(128 // CHUNK):
        u[c * CHUNK:(c + 1) * CHUNK, c * CHUNK:(c + 1) * CHUNK] = tri

    gWs = n1w[:, None] * gW
    vWs = n1w[:, None] * vW
    wgv = np.concatenate(
        [gWs[:, 0:512], vWs[:, 0:512], gWs[:, 512:768], vWs[:, 512:768]], axis=1)
    arrs = {
        "wgv": np.ascontiguousarray(wgv.reshape(KD, 128, 2 * D).astype(bf16)),
        "w1": np.ascontiguousarray((n2w[:, None] * W1).reshape(KD, 128, H).astype(bf16)),
        "w2": np.ascontiguousarray((0.5 * W2).reshape(KH, 128, D).astype(bf16)),
        "u": u.astype(bf16),
        "idb": np.eye(128, dtype=bf16),
        "idf": np.eye(128, dtype=f32),
    }
    if flags[0] or flags[1]:
        arrs["bgv"] = np.concatenate(
            [bg[:, 0:512], bv[:, 0:512], bg[:, 512:768], bv[:, 512:768]], axis=1)
    if flags[2]:
        arrs["b1"] = b1f
    if flags[3]:
        arrs["b2"] = b2f
    return arrs, flags


_CACHE: dict = {}


def _get_exec(flags):
    """Build (once) the Bass module and a cached jitted PJRT executable."""
    if _CACHE.get("flags") == flags:
        return _CACHE
    import jax
    from concourse import bass2jax
    from concourse import mybir
    from concourse.bass2jax import (
        Mesh, PartitionSpec, shard_map, _bass_exec_p, install_neuronx_cc_hook,
        partition_id_tensor,
    )

    nc = build(TPC, True, *flags)
    install_neuronx_cc_hook()
    assert nc.dbg_addr is None
    partition_name = nc.partition_id_tensor.name if nc.partition_id_tensor else None

    in_names, out_names, out_avals, zero_outs = [], [], [], []
    for alloc in nc.m.functions[0].allocations:
        if not isinstance(alloc, mybir.MemoryLocationSet):
            continue
        name = alloc.memorylocations[0].name
        if alloc.kind == "ExternalInput":
            if name != partition_name:
                in_names.append(name)
        elif alloc.kind == "ExternalOutput":
            shape = tuple(alloc.tensor_shape)
            dtype = mybir.dt.np(alloc.dtype)
            out_names.append(name)
            out_avals.append(jax.core.ShapedArray(shape, dtype))
            zero_outs.append(np.zeros(shape, dtype))
    n_params = len(in_names)
    n_outs = len(out_avals)
    all_names = in_names + out_names
    if partition_name is not None:
        all_names = all_names + [partition_name]
    donate = tuple(range(n_params, n_params + n_outs))

    def _body(*args):
        operands = list(args)
        if partition_name is not None:
            operands.append(partition_id_tensor())
        outs = _bass_exec_p.bind(
            *operands,
            out_avals=tuple(out_avals),
            in_names=tuple(all_names),
            out_names=tuple(out_names),
            lowering_input_output_aliases=(),
            sim_require_finite=True,
            sim_require_nnan=True,
            nc=nc,
        )
        return tuple(outs)

    devices = jax.devices()[:NCORES]
    assert len(devices) == NCORES
    mesh = Mesh(np.asarray(devices), ("core",))
    sharded = jax.jit(
        shard_map(_body, mesh=mesh, in_specs=(PartitionSpec("core"),) * (n_params + n_outs),
                  out_specs=(PartitionSpec("core"),) * n_outs, check_rep=False),
        donate_argnums=donate, keep_unused=True,
    )
    _CACHE.clear()
    _CACHE.update(
        flags=flags, nc=nc, sharded=sharded, in_names=in_names,
        out_names=out_names, out_avals=out_avals, zero_outs=zero_outs, mesh=mesh,
    )
    return _CACHE


def _run(arrs, flags, x_flat):
    st = _get_exec(flags)
    concat_in = []
    for name in st["in_names"]:
        if name == "x":
            concat_in.append(np.ascontiguousarray(x_flat))
        else:
            a = arrs[name]
            concat_in.append(np.concatenate([a] * NCORES, axis=0))
    concat_zeros = [
        np.zeros((NCORES * z.shape[0], *z.shape[1:]), z.dtype) for z in st["zero_outs"]
    ]
    out_arrs = st["sharded"](*concat_in, *concat_zeros)
    i = st["out_names"].index("outT")
    o = np.asarray(out_arrs[i]).reshape(NCORES, D, TPC)
    return o


def kernel(**inputs):
    x = np.asarray(inputs["x"], np.float32).reshape(TOTAL, D)
    arrs, flags = _fold(inputs)
    try:
        o = _run(arrs, flags, x)
        parts = [o[c].T for c in range(NCORES)]
    except Exception:
        from concourse.bass_utils import run_bass_kernel_spmd
        if _CACHE.get("flags") != flags or "nc" not in _CACHE:
            _CACHE.clear()
            _CACHE["nc"] = build(TPC, True, *flags)
            _CACHE["flags"] = flags
        in_maps = [
            {**arrs, "x": np.ascontiguousarray(x[c * TPC:(c + 1) * TPC])}
            for c in range(NCORES)
        ]
        res = run_bass_kernel_spmd(_CACHE["nc"], in_maps, list(range(NCORES)),
                                   trace=False)
        parts = [np.asarray(res.results[c]["outT"]).T for c in range(NCORES)]
    return np.concatenate(parts, axis=0).reshape(B, S, D).astype(np.float32)



# revision 26
# speedup vs baseline: 96.3350x; 96.3350x over previous
"""Trainium2 Bass kernel: PSI block (LN1 -> sigmoid-gated value -> chunked
normalized cumsum -> residual -> LN2 -> exact-gelu FFN -> residual).

Sharding: 32768 tokens split into 8 contiguous 4096-token shards (chunk- and
batch-boundary aligned), one per NeuronCore; dim-sized weights replicated.

Per core, two passes (x2 stays resident in SBUF as bf16 -- no DRAM scratch):
  pass A (32 token-tiles of 128): LN1 stats (ACT Square accum + DVE reduce,
    Newton-iteration rsqrt on DVE), z.T built by PE transposes, bf16
    gate/value matmuls into three single-bank PSUM tiles, sigmoid, chunked
    cumsum via block-triangular matmul, mem = csum(g*v)/(csum(g)+1e-6),
    x2 = x + mem -> bf16 SBUF, LN2 stats stored for pass B.
  pass B (16 macros of 256 tokens): h2.T via PE transposes, FFN1 (bf16),
    exact gelu via Erf, FFN2 with NON-transposed [tok, d] PSUM accumulation
    (stationary = gelu-activation blocks, moving = W2 rows), residual added
    on DVE from the bf16 SBUF x2, natural-layout [T, D] f32 output.
"""

import sys

sys.path.insert(0, "/opt/trn_rl_repo")

import numpy as np
import ml_dtypes
from contextlib import ExitStack

B, S, D, CHUNK = 4, 8192, 768, 64
NCORES = 8
TOTAL = B * S              # 32768 tokens
TPC = TOTAL // NCORES      # 4096 tokens per core
KD = D // 128              # 6 k-blocks over D
H = 4 * D                  # 3072 FFN hidden
KH = H // 128              # 24 k-blocks over H
MACRO = 256                # pass-B token macro (2 tiles)
INV_SQRT2 = 0.7071067811865476


def build(T=TPC, erf_ok=True, gbias=False, vbias=False, f1bias=False, f2bias=False,
          reps=1):
    import concourse.bass as bass
    import concourse.bacc as bacc
    import concourse.tile as tile
    from concourse import mybir

    F32 = mybir.dt.float32
    BF16 = mybir.dt.bfloat16
    I32 = mybir.dt.int32
    AF = mybir.ActivationFunctionType
    ALU = mybir.AluOpType
    PSUM = bass.MemorySpace.PSUM
    ts = bass.ts

    NT = T // 128
    NM = T // MACRO
    NS = MACRO // 128
    any_bias = gbias or vbias or f1bias or f2bias

    nc = bacc.Bacc(None, target_bir_lowering=False, debug=False)

    gv_b = gbias or vbias
    x_d = nc.dram_tensor("x", [T, D], F32, kind="ExternalInput")
    wgv_d = nc.dram_tensor("wgv", [128, KD * 2 * D], BF16, kind="ExternalInput")
    w1_d = nc.dram_tensor("w1", [128, KD * H], BF16, kind="ExternalInput")
    w2_d = nc.dram_tensor("w2", [128, KH * D], BF16, kind="ExternalInput")
    u_d = nc.dram_tensor("u", [128, 128], BF16, kind="ExternalInput")
    bgv_d = nc.dram_tensor("bgv", [1, 2 * D], BF16, kind="ExternalInput") if gv_b else None
    b1_d = nc.dram_tensor("b1", [1, H], BF16, kind="ExternalInput") if f1bias else None
    b2_d = nc.dram_tensor("b2", [1, D], BF16, kind="ExternalInput") if f2bias else None
    out_d = nc.dram_tensor("out", [T, D], F32, kind="ExternalOutput")

    with tile.TileContext(nc) as tc, ExitStack() as ctx:
        const = ctx.enter_context(tc.tile_pool(name="const", bufs=1))
        u_sb = const.tile([128, 128], BF16, tag="u")
        nc.sync.dma_start(u_sb[:], u_d[:])
        # wgv is needed first (pass-A matmuls); w1/w2 DMAs are issued lazily in
        # chunks from inside pass A so they don't sit ahead of the x-tile loads
        # on the DMA engines.
        wgv_sb = const.tile([128, KD * 2 * D], BF16, tag="wgv")
        w1_sb = const.tile([128, KD * H], BF16, tag="w1")
        w2_sb = const.tile([128, KH * D], BF16, tag="w2")
        x2_sb = const.tile([128, NT * D], BF16, tag="x2")
        rstd2_all = const.tile([128, NT], F32, tag="rstd2")
        nmr2_all = const.tile([128, NT], F32, tag="nmr2")
        sqs2_all = const.tile([128, NT], F32, tag="sqs2")
        xs2_all = const.tile([128, NT], F32, tag="xs2")
        if gv_b:
            bgv_sb = const.tile([1, 2 * D], BF16, tag="bgv")
            nc.sync.dma_start(bgv_sb[:], bgv_d[:])
        if f1bias:
            b1_sb = const.tile([1, H], BF16, tag="b1")
            nc.sync.dma_start(b1_sb[:], b1_d[:])
        if f2bias:
            b2_sb = const.tile([1, D], BF16, tag="b2")
            nc.sync.dma_start(b2_sb[:], b2_d[:])
        if any_bias:
            ones_sb = const.tile([1, MACRO], BF16, tag="ones")
            nc.vector.memset(ones_sb[:], 1.0)

        def newton_rsqrt(pool, tag, v, out_ap=None, w=1):
            """y ~ rsqrt(v) for v [128,w] f32 > 0; quake seed + 2 NR iters on DVE."""
            y = pool.tile([128, w], F32, tag=tag + "_y", bufs=2, name="y")
            a = pool.tile([128, w], F32, tag=tag + "_a", bufs=2, name="a")
            nc.vector.tensor_scalar(
                y[:].bitcast(I32), v[:].bitcast(I32), 1, -1,
                op0=ALU.logical_shift_right, op1=ALU.bitwise_xor,
            )
            nc.vector.tensor_scalar(
                y[:].bitcast(I32), y[:].bitcast(I32), 0x5F3759E0, None, op0=ALU.add
            )
            for it in range(2):
                nc.vector.tensor_mul(a[:], y[:], y[:])
                nc.vector.tensor_mul(a[:], a[:], v[:])
                nc.vector.tensor_scalar(a[:], a[:], -0.5, 1.5, op0=ALU.mult, op1=ALU.add)
                dst = out_ap if (it == 1 and out_ap is not None) else y[:]
                nc.vector.tensor_mul(dst, y[:], a[:])
            return y

        # ---------------- pass A ----------------
        def pass_a():
            with tc.tile_pool(name="pa", bufs=1) as pa, \
                    tc.tile_pool(name="psa", bufs=1, space=PSUM) as psa:
                run_pass_a(pa, psa)

        def run_pass_a(pa, psa):
            xs, hus, lnts = {}, {}, {}
            # lazy weight-chunk DMAs, issued behind the early x-tile loads
            wchunks = [(w1_sb, w1_d, k * H, (k + 1) * H) for k in range(KD)]
            wchunks += [(w2_sb, w2_d, 6 * D * c, 6 * D * (c + 1)) for c in range(4)]
            next_chunk = [0]

            def issue_chunk():
                if next_chunk[0] < len(wchunks):
                    dst, src, c0, c1 = wchunks[next_chunk[0]]
                    nc.sync.dma_start(dst[:, c0:c1], src[:, c0:c1])
                    next_chunk[0] += 1

            def stage_dma(t):
                x_sb = pa.tile([128, D], F32, tag="x", bufs=6, name="x_sb")
                nc.sync.dma_start(x_sb[:], x_d[128 * t:128 * (t + 1), :])
                xs[t] = x_sb
                if t >= 3:
                    issue_chunk()

            def stage_stats(t):
                """LN1 stats + normalized bf16 tile; runs 2 tiles ahead.

                Fused mean/var via DVE bn_stats (free dim capped at 512 ->
                two 384-wide groups) + bn_aggr.
                """
                x_sb = xs[t]
                bns = pa.tile([128, 2, 6], F32, tag="bns", bufs=2, name="bns")
                nc.vector.bn_stats(bns[:, 0, :], x_sb[:, 0:384])
                nc.vector.bn_stats(bns[:, 1, :], x_sb[:, 384:768])
                agg = pa.tile([128, 2], F32, tag="agg", bufs=2, name="agg")
                nc.vector.bn_aggr(agg[:], bns[:])
                v = pa.tile([128, 1], F32, tag="v1", bufs=2, name="v1")
                nc.vector.tensor_scalar(v[:], agg[:, 1:2], 1.0, 1e-5,
                                        op0=ALU.mult, op1=ALU.add)
                rstd = newton_rsqrt(pa, "n1", v)
                nmr1 = pa.tile([128, 1], F32, tag="nmr1", bufs=2, name="nmr1")
                nc.vector.scalar_tensor_tensor(nmr1[:], agg[:, 0:1], -1.0, rstd[:],
                                               op0=ALU.mult, op1=ALU.mult)
                # normalized x in bf16 on ACT: Identity(x * rstd + nmr1)
                hu = pa.tile([128, D], BF16, tag="hu", bufs=2, name="hu")
                nc.scalar.activation(hu[:], x_sb[:], AF.Identity,
                                     bias=nmr1[:], scale=rstd[:])
                hus[t] = hu

            def stage_lnT(t):
                # z.T via one XBAR DMA transpose (runs on the idle DMA engines;
                # no PE/PSUM involvement): lnT[p, k, c] = hu[c, 128k+p].
                hu = hus.pop(t)
                lnT = pa.tile([128, KD, 128], BF16, tag="lnT", bufs=2, name="lnT")
                nc.sync.dma_start_transpose(lnT[:], hu[:])
                lnts[t] = lnT

            def stage_stats2(t):
                """LN2 raw sums for tile t (batched rsqrt after the loop)."""
                x2b = x2_sb[:, D * t:D * (t + 1)]
                scr = pa.tile([128, D], BF16, tag="sqscr", bufs=2, name="sqscr")
                nc.scalar.activation(scr[:], x2b, AF.Square,
                                     accum_out=sqs2_all[:, t:t + 1])
                scr2 = pa.tile([128, D], BF16, tag="sqscr2", bufs=2, name="sqscr2")
                nc.scalar.activation(scr2[:], x2b, AF.Identity,
                                     accum_out=xs2_all[:, t:t + 1])

            def ln2_batch(lo, hi):
                """Batched LN2 scalar math for tiles [lo, hi)."""
                w = hi - lo
                nmu2 = pa.tile([128, w], F32, tag=f"nmu2_{lo}", bufs=1,
                               name="nmu2")
                nc.vector.tensor_scalar(nmu2[:], xs2_all[:, lo:hi], -1.0 / D,
                                        None, op0=ALU.mult)
                v2 = pa.tile([128, w], F32, tag=f"v2_{lo}", bufs=1, name="v2")
                nc.vector.tensor_scalar(v2[:], sqs2_all[:, lo:hi], 1.0 / D, 1e-5,
                                        op0=ALU.mult, op1=ALU.add)
                m22 = pa.tile([128, w], F32, tag=f"m22_{lo}", bufs=1, name="m22")
                nc.vector.tensor_mul(m22[:], nmu2[:], nmu2[:])
                nc.vector.tensor_sub(v2[:], v2[:], m22[:])
                newton_rsqrt(pa, f"n2_{lo}", v2, out_ap=rstd2_all[:, lo:hi], w=w)
                nc.vector.tensor_mul(nmr2_all[:, lo:hi], nmu2[:],
                                     rstd2_all[:, lo:hi])

            stage_dma(0)
            stage_dma(1)
            # wgv in three bank-column chunks so the first gate/value matmuls
            # can start as soon as chunk 0 lands
            wgv3 = wgv_sb[:].rearrange("p (k c) -> p k c", k=KD)
            wgv3_d = wgv_d[:].rearrange("p (k c) -> p k c", k=KD)
            nc.sync.dma_start(wgv3[:, :, 0:512], wgv3_d[:, :, 0:512])
            stage_stats(0)
            stage_stats(1)
            stage_lnT(0)
            nc.sync.dma_start(wgv3[:, :, 512:1024], wgv3_d[:, :, 512:1024])
            nc.sync.dma_start(wgv3[:, :, 1024:1536], wgv3_d[:, :, 1024:1536])
            stage_dma(2)
            for t in range(NT):
                x_sb = xs.pop(t)
                lnT = lnts.pop(t)
                # merged gate|value pre-activations in three single-bank PSUM
                # tiles (finer WAR granularity than one 3-bank tile): wgv
                # columns are [g 0:512 | v 0:512 | g 512:768 | v 512:768]
                pgv = []
                for bb in range(3):
                    p = psa.tile([128, 512], F32, tag=f"pgv{bb}", bufs=1,
                                 name=f"pgv{bb}")
                    mm = [(lnT[:, k, :],
                           wgv_sb[:, k * 2 * D + 512 * bb:k * 2 * D + 512 * (bb + 1)])
                          for k in range(KD)]
                    if gv_b:
                        mm.append((ones_sb[0:1, 0:128],
                                   bgv_sb[0:1, 512 * bb:512 * (bb + 1)]))
                    for i, (l, r) in enumerate(mm):
                        nc.tensor.matmul(p[:], l, r,
                                         start=(i == 0), stop=(i == len(mm) - 1))
                    pgv.append(p)
                # gvg packs [g 0:768 | g*v 0:768] (original column order) so the
                # cumsum is 3 full-width matmuls and cs reads stay bank-aligned
                gvg = pa.tile([128, 2 * D], BF16, tag="gvg", bufs=2, name="gvg")
                nc.scalar.activation(gvg[:, 0:512], pgv[0][:], AF.Sigmoid)
                nc.scalar.activation(gvg[:, 512:768], pgv[2][:, 0:256], AF.Sigmoid)
                nc.vector.tensor_mul(gvg[:, 768:1280], gvg[:, 0:512], pgv[1][:])
                nc.vector.tensor_mul(gvg[:, 1280:1536], gvg[:, 512:768],
                                     pgv[2][:, 256:512])
                # deferred LN2 sums of the previous tile (ACT slack window)
                if t >= 1:
                    stage_stats2(t - 1)
                    if t - 1 == NT - 5 and NT > 8:
                        # early LN2 batch so pass B isn't gated on the finale
                        ln2_batch(0, NT - 4)
                # prefetch + LN1 stats for upcoming tiles while PE chews on t
                if t + 3 < NT:
                    stage_dma(t + 3)
                if t + 2 < NT:
                    stage_stats(t + 2)
                # transposes for t+1 issued between pgv(t) and cs(t) so the PE
                # stays busy while ACT/DVE produce gvg(t)
                if t + 1 < NT:
                    stage_lnT(t + 1)
                # chunked cumsum along tokens (partition dim) via triangular
                # matmul; cs = [csum(g) 0:768 | csum(g*v) 0:768]
                cs = []
                for bb in range(3):
                    c = psa.tile([128, 512], F32, tag=f"cs{bb}", bufs=1,
                                 name=f"cs{bb}")
                    nc.tensor.matmul(c[:], u_sb[:],
                                     gvg[:, 512 * bb:512 * (bb + 1)],
                                     start=True, stop=True)
                    cs.append(c)
                # reciprocal straight off PSUM; the reference's +1e-6 in the
                # denominator is dropped -- csum(g) >= sigmoid(preact) >~ 2e-3,
                # so the relative difference is < 5e-4. mem/x2 element-wise ops
                # run on the otherwise-idle GpSimd engine to unload the DVE.
                rcp = pa.tile([128, D], F32, tag="rcp", bufs=2, name="rcp")
                mem = pa.tile([128, D], F32, tag="mem", bufs=2, name="mem")
                nc.vector.reciprocal_approx_fast(rcp[:, 0:512], cs[0][:])
                nc.vector.reciprocal_approx_fast(rcp[:, 512:768], cs[1][:, 0:256])
                nc.vector.tensor_mul(mem[:, 0:256], rcp[:, 0:256], cs[1][:, 256:512])
                nc.vector.tensor_mul(mem[:, 256:768], rcp[:, 256:768], cs[2][:])
                x2b = x2_sb[:, D * t:D * (t + 1)]
                nc.gpsimd.tensor_add(x2b, x_sb[:], mem[:])
            stage_stats2(NT - 1)
            while next_chunk[0] < len(wchunks):
                issue_chunk()
            if NT > 8:
                ln2_batch(NT - 4, NT)
            else:
                ln2_batch(0, NT)

        # ---------------- pass B ----------------
        def pass_b():
            with tc.tile_pool(name="pb", bufs=1) as pb, \
                    tc.tile_pool(name="psb", bufs=1, space=PSUM) as psb:
                run_pass_b(pb, psb)

        def run_pass_b(pb, psb):
            for m in range(NM):
                h2T = pb.tile([128, KD, MACRO], BF16, tag="h2T", bufs=2, name="h2T")
                outp = []
                for s in range(NS):
                    tm = m * NS + s
                    h2s = pb.tile([128, D], BF16, tag="h2s", bufs=2, name="h2s")
                    nc.scalar.activation(h2s[:], x2_sb[:, D * tm:D * (tm + 1)],
                                         AF.Identity, bias=nmr2_all[:, tm:tm + 1],
                                         scale=rstd2_all[:, tm:tm + 1])
                    nc.sync.dma_start_transpose(h2T[:, :, 128 * s:128 * (s + 1)],
                                                h2s[:])
                    outp.append(psb.tile([128, 1024], F32, tag=f"out{s}", bufs=1,
                                         name=f"out{s}"))
                # software-pipelined: FFN1(f) is issued before FFN2(f-1) so the
                # gelu/uT of block f computes while PE runs FFN2 of block f-1.
                prev_uT = None
                for f in range(KH + 1):
                    if f < KH:
                        pT = psb.tile([128, 512], F32, tag="pt", bufs=4, name="pT")
                        mm = [(w1_sb[:, k * H + 128 * f:k * H + 128 * (f + 1)],
                               h2T[:, k, :])
                              for k in range(KD)]
                        if f1bias:
                            mm.append((b1_sb[0:1, 128 * f:128 * (f + 1)],
                                       ones_sb[0:1, 0:MACRO]))
                        for i, (l, r) in enumerate(mm):
                            nc.tensor.matmul(pT[:, 0:MACRO], l, r,
                                             start=(i == 0), stop=(i == len(mm) - 1))
                    if f >= 1:
                        fp = f - 1
                        for s in range(NS):
                            lhs = prev_uT[:, 128 * s:128 * (s + 1)]
                            nc.tensor.matmul(outp[s][:, 0:512], lhs,
                                             w2_sb[:, fp * D:fp * D + 512],
                                             start=(fp == 0), stop=(fp == KH - 1))
                            nc.tensor.matmul(outp[s][:, 512:768], lhs,
                                             w2_sb[:, fp * D + 512:fp * D + 768],
                                             start=(fp == 0), stop=(fp == KH - 1))
                        if fp == 0 and f2bias:
                            for s in range(NS):
                                nc.tensor.matmul(outp[s][:, 0:512],
                                                 ones_sb[0:1, 0:128],
                                                 b2_sb[0:1, 0:512],
                                                 start=False, stop=False)
                                nc.tensor.matmul(outp[s][:, 512:768],
                                                 ones_sb[0:1, 0:128],
                                                 b2_sb[0:1, 512:768],
                                                 start=False, stop=False)
                    if f < KH:
                        e_sb = pb.tile([128, MACRO], BF16, tag="e", bufs=2, name="e_sb")
                        nc.scalar.activation(e_sb[:], pT[:, 0:MACRO],
                                             AF.Erf if erf_ok else AF.Tanh,
                                             scale=INV_SQRT2)
                        uT = pb.tile([128, MACRO], BF16, tag="uT", bufs=3, name="uT")
                        nc.vector.scalar_tensor_tensor(uT[:], e_sb[:], 1.0,
                                                       pT[:, 0:MACRO],
                                                       op0=ALU.add, op1=ALU.mult)
                        prev_uT = uT
                for s in range(NS):
                    tm = m * NS + s
                    osb = pb.tile([128, D], F32, tag="osb", bufs=2, name="osb")
                    nc.vector.tensor_add(osb[:], outp[s][:, 0:768],
                                         x2_sb[:, D * tm:D * (tm + 1)])
                    nc.sync.dma_start(out_d[128 * tm:128 * (tm + 1), :], osb[:])

        for _ in range(reps):
            pass_a()
            pass_b()

    nc.compile()
    return nc


def _fold(inputs):
    f32 = np.float32
    bf16 = ml_dtypes.bfloat16
    n1w = np.asarray(inputs["norm1_w"], f32)
    n1b = np.asarray(inputs["norm1_b"], f32)
    n2w = np.asarray(inputs["norm2_w"], f32)
    n2b = np.asarray(inputs["norm2_b"], f32)
    gW = np.asarray(inputs["gate_W"], f32)
    gb = np.asarray(inputs["gate_b"], f32)
    vW = np.asarray(inputs["value_W"], f32)
    vb = np.asarray(inputs["value_b"], f32)
    W1 = np.asarray(inputs["ffn_W1"], f32)
    b1 = np.asarray(inputs["ffn_b1"], f32)
    W2 = np.asarray(inputs["ffn_W2"], f32)
    b2 = np.asarray(inputs["ffn_b2"], f32)

    bg = (n1b @ gW + gb).astype(bf16).reshape(1, D)
    bv = (n1b @ vW + vb).astype(bf16).reshape(1, D)
    b1f = (n2b @ W1 + b1).astype(bf16).reshape(1, H)
    b2f = b2.astype(bf16).reshape(1, D)
    flags = (bool(bg.any()), bool(bv.any()), bool(b1f.any()), bool(b2f.any()))

    tri = np.triu(np.ones((CHUNK, CHUNK), f32))
    u = np.zeros((128, 128), f32)
    for c in range(128 // CHUNK):
        u[c * CHUNK:(c + 1) * CHUNK, c * CHUNK:(c + 1) * CHUNK] = tri

    def kpack(a, kb):
        """[kb*128, N] -> [128, kb*N] with per-partition [kb, N] blocks;
        contraction row kb*128+p sits in block kb at partition p."""
        n = a.shape[1]
        return np.ascontiguousarray(
            a.reshape(kb, 128, n).transpose(1, 0, 2).reshape(128, kb * n))

    gWs = n1w[:, None] * gW
    vWs = n1w[:, None] * vW
    wgv = np.concatenate(
        [gWs[:, 0:512], vWs[:, 0:512], gWs[:, 512:768], vWs[:, 512:768]], axis=1)
    arrs = {
        "wgv": kpack(wgv, KD).astype(bf16),
        "w1": kpack(n2w[:, None] * W1, KD).astype(bf16),
        "w2": kpack(0.5 * W2, KH).astype(bf16),
        "u": u.astype(bf16),
    }
    if flags[0] or flags[1]:
        arrs["bgv"] = np.concatenate(
            [bg[:, 0:512], bv[:, 0:512], bg[:, 512:768], bv[:, 512:768]], axis=1)
    if flags[2]:
        arrs["b1"] = b1f
    if flags[3]:
        arrs["b2"] = b2f
    return arrs, flags


_CACHE: dict = {}


def _get_exec(flags):
    """Build (once) the Bass module and a cached jitted PJRT executable."""
    if _CACHE.get("flags") == flags:
        return _CACHE
    import jax
    from concourse import mybir
    from concourse.bass2jax import (
        Mesh, PartitionSpec, shard_map, _bass_exec_p, install_neuronx_cc_hook,
        partition_id_tensor,
    )

    nc = build(TPC, True, *flags)
    install_neuronx_cc_hook()
    assert nc.dbg_addr is None
    partition_name = nc.partition_id_tensor.name if nc.partition_id_tensor else None

    in_names, out_names, out_avals, zero_outs = [], [], [], []
    for alloc in nc.m.functions[0].allocations:
        if not isinstance(alloc, mybir.MemoryLocationSet):
            continue
        name = alloc.memorylocations[0].name
        if alloc.kind == "ExternalInput":
            if name != partition_name:
                in_names.append(name)
        elif alloc.kind == "ExternalOutput":
            shape = tuple(alloc.tensor_shape)
            dtype = mybir.dt.np(alloc.dtype)
            out_names.append(name)
            out_avals.append(jax.core.ShapedArray(shape, dtype))
            zero_outs.append(np.zeros(shape, dtype))
    n_params = len(in_names)
    n_outs = len(out_avals)
    all_names = in_names + out_names
    if partition_name is not None:
        all_names = all_names + [partition_name]
    donate = tuple(range(n_params, n_params + n_outs))

    def _body(*args):
        operands = list(args)
        if partition_name is not None:
            operands.append(partition_id_tensor())
        outs = _bass_exec_p.bind(
            *operands,
            out_avals=tuple(out_avals),
            in_names=tuple(all_names),
            out_names=tuple(out_names),
            lowering_input_output_aliases=(),
            sim_require_finite=True,
            sim_require_nnan=True,
            nc=nc,
        )
        return tuple(outs)

    devices = jax.devices()[:NCORES]
    assert len(devices) == NCORES
    mesh = Mesh(np.asarray(devices), ("core",))
    sharded = jax.jit(
        shard_map(_body, mesh=mesh, in_specs=(PartitionSpec("core"),) * (n_params + n_outs),
                  out_specs=(PartitionSpec("core"),) * n_outs, check_rep=False),
        donate_argnums=donate, keep_unused=True,
    )
    _CACHE.clear()
    _CACHE.update(
        flags=flags, nc=nc, sharded=sharded, in_names=in_names,
        out_names=out_names, out_avals=out_avals, zero_outs=zero_outs, mesh=mesh,
    )
    return _CACHE


def _run(arrs, flags, x_flat):
    st = _get_exec(flags)
    concat_in = []
    for name in st["in_names"]:
        if name == "x":
            concat_in.append(np.ascontiguousarray(x_flat))
        else:
            a = arrs[name]
            concat_in.append(np.concatenate([a] * NCORES, axis=0))
    concat_zeros = [
        np.zeros((NCORES * z.shape[0], *z.shape[1:]), z.dtype) for z in st["zero_outs"]
    ]
    out_arrs = st["sharded"](*concat_in, *concat_zeros)
    i = st["out_names"].index("out")
    o = np.asarray(out_arrs[i]).reshape(NCORES, TPC, D)
    return o


def kernel(**inputs):
    x = np.asarray(inputs["x"], np.float32).reshape(TOTAL, D)
    arrs, flags = _fold(inputs)
    try:
        o = _run(arrs, flags, x)
    except Exception:
        from concourse.bass_utils import run_bass_kernel_spmd
        if _CACHE.get("flags") != flags or "nc" not in _CACHE:
            _CACHE.clear()
            _CACHE["nc"] = build(TPC, True, *flags)
            _CACHE["flags"] = flags
        in_maps = [
            {**arrs, "x": np.ascontiguousarray(x[c * TPC:(c + 1) * TPC])}
            for c in range(NCORES)
        ]
        res = run_bass_kernel_spmd(_CACHE["nc"], in_maps, list(range(NCORES)),
                                   trace=False)
        o = np.stack([np.asarray(res.results[c]["out"]) for c in range(NCORES)])
    return o.reshape(B, S, D).astype(np.float32)
